# revision 1
# baseline (speedup 1.0000x reference)
"""Multi-head attention (B=2, T=2048, D=2048, 16 heads) on 8 NeuronCores.

Sharding: DP=2 over batch x TP=4 over heads (4 heads/core).
Core c handles batch b=c//4, head group r=c%4 (heads 4r..4r+3).

Per-core dataflow (all matmuls in float32r, single-pass FP22 on PE):
  P1: Q^T, K^T (dh-on-partitions) and V (tokens-on-partitions) projections.
      Host passes x[b]^T and W^T slices so every matmul operand is in its
      natural layout -- no on-device transposes anywhere.
  P2: per head: S^T = K_h^T^T@Q_h^T chunks -> exp (ScalarE, scaled 1/sqrt(dh))
      -> PV accumulation (attn^T in PSUM) with column sums via a ones-matmul;
      normalize with DVE using a DMA-broadcast reciprocal.
  P3: AllGather attn^T over the 4-core batch group, then each core computes
      its 512 output columns: out = attn_full @ Wo^T[:, cols].

Output per core: (2048 tokens, 512 out-cols); host concatenates.
"""

import math

import numpy as np

import concourse.bass as bass
import concourse.mybir as mybir
import concourse.tile as tile
from concourse import bacc
from concourse.bass_utils import run_bass_kernel_spmd

D = 2048
T = 2048
HG = 4  # heads per core
DH = 128
NI = 16  # contraction chunks of 128 over D
NQ = 4  # query-token chunks of 512
NT = 16  # token chunks of 128
SCALE = 1.0 / math.sqrt(DH)
F32 = mybir.dt.float32
F32R = mybir.dt.float32r
GROUPS = [[0, 1, 2, 3], [4, 5, 6, 7]]

_CACHED = {}


def build():
    nc = bacc.Bacc("TRN2", target_bir_lowering=False, debug=False, num_devices=8)
    xT = nc.declare_dram_parameter("xT", [D, T], F32R, isOutput=False)
    wqT = nc.declare_dram_parameter("wqT", [D, HG * DH], F32R, isOutput=False)
    wkT = nc.declare_dram_parameter("wkT", [D, HG * DH], F32R, isOutput=False)
    wvT = nc.declare_dram_parameter("wvT", [D, HG * DH], F32R, isOutput=False)
    woT = nc.declare_dram_parameter("woT", [D, HG * DH], F32R, isOutput=False)
    out = nc.declare_dram_parameter("out", [T, HG * DH], F32, isOutput=True)

    with tile.TileContext(nc) as tc:
        with (
            tc.tile_pool(name="dram", bufs=1, space="DRAM") as dram,
            tc.tile_pool(name="keep", bufs=1) as keep,
        ):
            attn_mine = dram.tile([HG * DH, T], F32R)
            attn_all = dram.tile([4 * HG * DH, T], F32R)
            qT_d = dram.tile([HG * DH, T], F32R)
            kT_d = dram.tile([HG * DH, T], F32R)

            v_sb = keep.tile([128, NT, HG * DH], F32R)  # V: [tok128, tchunk, hdims]
            ones_f32 = keep.tile([128, 1], F32)
            nc.vector.memset(ones_f32[:], 1.0)
            ones_sb = keep.tile([128, 1], F32R)
            nc.vector.tensor_copy(ones_sb[:], ones_f32[:])

            # ---------------- Phase 1: QKV projections ----------------
            with (
                tc.tile_pool(name="p1x", bufs=1) as p1x,
                tc.tile_pool(name="p1w", bufs=1) as p1w,
                tc.tile_pool(name="p1s", bufs=6) as p1s,
                tc.tile_pool(name="p1p", bufs=4, space="PSUM") as p1p,
            ):
                x_sb = p1x.tile([128, NI, T], F32R)  # x^T resident: 128KB/part
                for i in range(NI):
                    nc.sync.dma_start(
                        out=x_sb[:, i, :], in_=xT[i * 128 : (i + 1) * 128, :]
                    )

                # Q^T and K^T: out rows = head dims (M), moving = tokens
                for w_par, dst in ((wqT, qT_d), (wkT, kT_d)):
                    w_sb = p1w.tile([128, NI, HG * DH], F32R, tag="w_sb")
                    for i in range(NI):
                        nc.sync.dma_start(
                            out=w_sb[:, i, :], in_=w_par[i * 128 : (i + 1) * 128, :]
                        )
                    for m in range(HG):
                        psums = []
                        for t in range(NQ):
                            psums.append(
                                p1p.tile([128, 512], F32, name="qk_ps", tag="qk_ps")
                            )
                        for i in range(NI):
                            lhsT = w_sb[:, i, m * 128 : (m + 1) * 128]
                            for t in range(NQ):
                                nc.tensor.matmul(
                                    psums[t][:],
                                    lhsT,
                                    x_sb[:, i, t * 512 : (t + 1) * 512],
                                    start=(i == 0),
                                    stop=(i == NI - 1),
                                )
                        for t in range(NQ):
                            st = p1s.tile([128, 512], F32R)
                            nc.vector.tensor_copy(st[:], psums[t][:])
                            nc.sync.dma_start(
                                out=dst[
                                    m * 128 : (m + 1) * 128, t * 512 : (t + 1) * 512
                                ],
                                in_=st[:],
                            )

                # V: natural layout, tokens = M (stationary = x^T chunk)
                w_sb = p1w.tile([128, NI, HG * DH], F32R, tag="w_sb")
                for i in range(NI):
                    nc.sync.dma_start(
                        out=w_sb[:, i, :], in_=wvT[i * 128 : (i + 1) * 128, :]
                    )
                for tc_i in range(NT):
                    ps = p1p.tile([128, 512], F32)
                    for i in range(NI):
                        nc.tensor.matmul(
                            ps[:],
                            x_sb[:, i, tc_i * 128 : (tc_i + 1) * 128],
                            w_sb[:, i, :],
                            start=(i == 0),
                            stop=(i == NI - 1),
                        )
                    nc.vector.tensor_copy(v_sb[:, tc_i, :], ps[:])

            # ---------------- Phase 2: attention per head ----------------
            with (
                tc.tile_pool(name="p2qk", bufs=2) as p2qk,
                tc.tile_pool(name="p2e", bufs=4) as p2e,
                tc.tile_pool(name="p2a", bufs=2) as p2a,
                tc.tile_pool(name="p2n", bufs=2) as p2n,
                tc.tile_pool(name="p2ps", bufs=3, space="PSUM") as p2ps,
                tc.tile_pool(name="p2pa", bufs=2, space="PSUM") as p2pa,
                tc.tile_pool(name="p2pc", bufs=2, space="PSUM") as p2pc,
            ):
                for h in range(HG):
                    qh = p2qk.tile([128, T], F32R, tag="qh")
                    kh = p2qk.tile([128, T], F32R, tag="kh")
                    nc.sync.dma_start(out=qh[:], in_=qT_d[h * 128 : (h + 1) * 128, :])
                    nc.sync.dma_start(out=kh[:], in_=kT_d[h * 128 : (h + 1) * 128, :])
                    for q in range(NQ):
                        acc = p2a.tile([128, 512], F32R, tag="acc")
                        attn_ps = p2pa.tile([128, 512], F32, tag="attn_ps")
                        for k in range(NT):
                            s_ps = p2ps.tile([128, 512], F32, tag="s_ps")
                            nc.tensor.matmul(
                                s_ps[:],
                                kh[:, k * 128 : (k + 1) * 128],
                                qh[:, q * 512 : (q + 1) * 512],
                            )
                            expS = p2e.tile([128, 512], F32R, tag="expS")
                            nc.scalar.activation(
                                expS[:],
                                s_ps[:],
                                mybir.ActivationFunctionType.Exp,
                                scale=SCALE,
                            )
                            if k == 0:
                                nc.vector.tensor_copy(acc[:], expS[:])
                            else:
                                nc.vector.tensor_add(acc[:], acc[:], expS[:])
                            nc.tensor.matmul(
                                attn_ps[:],
                                v_sb[:, k, h * 128 : (h + 1) * 128],
                                expS[:],
                                start=(k == 0),
                                stop=(k == NT - 1),
                            )
                        csum = p2pc.tile([1, 512], F32, tag="csum")
                        nc.tensor.matmul(
                            csum[:], ones_sb[:], acc[:]
                        )
                        recip = p2n.tile([1, 512], F32, tag="recip")
                        nc.vector.reciprocal(recip[:], csum[:])
                        recip_d = dram.tile(
                            [1, 512], F32, name="recip_d", tag="recip_d", bufs=2
                        )
                        nc.sync.dma_start(out=recip_d[:], in_=recip[:])
                        bc = p2n.tile([128, 512], F32, tag="bc")
                        bcast_src = bass.AP(
                            tensor=recip_d.tensor,
                            offset=recip_d.offset,
                            ap=[[0, 128]] + [list(x) for x in recip_d.ap[1:]],
                        )
                        nc.sync.dma_start(out=bc[:], in_=bcast_src)
                        attn_sb = p2a.tile([128, 512], F32R, tag="attn_sb")
                        nc.vector.tensor_mul(attn_sb[:], attn_ps[:], bc[:])
                        nc.sync.dma_start(
                            out=attn_mine[
                                h * 128 : (h + 1) * 128, q * 512 : (q + 1) * 512
                            ],
                            in_=attn_sb[:],
                        )

            # ---------------- AllGather over batch group ----------------
            nc.gpsimd.collective_compute(
                "AllGather",
                mybir.AluOpType.bypass,
                replica_groups=GROUPS,
                ins=[attn_mine.opt()],
                outs=[attn_all.opt()],
            )

            # ---------------- Phase 3: output projection ----------------
            with (
                tc.tile_pool(name="p3w", bufs=1) as p3w,
                tc.tile_pool(name="p3a", bufs=8) as p3a,
                tc.tile_pool(name="p3o", bufs=4) as p3o,
                tc.tile_pool(name="p3p", bufs=4, space="PSUM") as p3p,
            ):
                wo_sb = p3w.tile([128, NI, HG * DH], F32R)
                for i in range(NI):
                    nc.sync.dma_start(
                        out=wo_sb[:, i, :], in_=woT[i * 128 : (i + 1) * 128, :]
                    )
                for t in range(NT):
                    ps = p3p.tile([128, 512], F32)
                    for i in range(NI):
                        a_tile = p3a.tile([128, 128], F32R, tag="a_tile")
                        nc.sync.dma_start(
                            out=a_tile[:],
                            in_=attn_all[
                                i * 128 : (i + 1) * 128, t * 128 : (t + 1) * 128
                            ],
                        )
                        nc.tensor.matmul(
                            ps[:],
                            a_tile[:],
                            wo_sb[:, i, :],
                            start=(i == 0),
                            stop=(i == NI - 1),
                        )
                    o_sb = p3o.tile([128, 512], F32)
                    nc.vector.tensor_copy(o_sb[:], ps[:])
                    nc.sync.dma_start(
                        out=out[t * 128 : (t + 1) * 128, :], in_=o_sb[:]
                    )

    nc.compile()
    return nc


def _get_nc():
    if "nc" not in _CACHED:
        _CACHED["nc"] = build()
    return _CACHED["nc"]


def kernel(x, Wq, Wk, Wv, Wo, _trace=False):
    x = np.asarray(x, dtype=np.float32)
    Wq = np.asarray(Wq, dtype=np.float32)
    Wk = np.asarray(Wk, dtype=np.float32)
    Wv = np.asarray(Wv, dtype=np.float32)
    Wo = np.asarray(Wo, dtype=np.float32)
    B = x.shape[0]

    in_maps = []
    for c in range(8):
        b, r = divmod(c, 4)
        sl = slice(r * 512, (r + 1) * 512)
        in_maps.append(
            {
                "xT": np.ascontiguousarray(x[b].T),
                "wqT": np.ascontiguousarray(Wq[sl, :].T),
                "wkT": np.ascontiguousarray(Wk[sl, :].T),
                "wvT": np.ascontiguousarray(Wv[sl, :].T),
                "woT": np.ascontiguousarray(Wo[sl, :].T),
            }
        )

    nc = _get_nc()
    res = run_bass_kernel_spmd(nc, in_maps, list(range(8)), trace=_trace)
    _CACHED["last_result"] = res

    out = np.empty((B, T, D), dtype=np.float32)
    for c in range(8):
        b, r = divmod(c, 4)
        out[b, :, r * 512 : (r + 1) * 512] = res.results[c]["out"]
    return out



# revision 3
# speedup vs baseline: 5.7786x; 5.7786x over previous
"""Multi-head attention (B=2, T=2048, D=2048, 16 heads) on 8 NeuronCores.

Sharding: DP=2 over batch x TP=4 over heads (4 heads/core).
Core c handles batch b=c//4, head group r=c%4 (heads 4r..4r+3).

Host->device traffic is the bottleneck (axon tunnel ~60MB/s up, ~30MB/s
down), so inputs are shipped fp16 and fully deduplicated: each core
uploads a disjoint 1/8th of x (512 rows of x[b]^T) and disjoint quarter
column-slices of each weight.  Two on-device AllGathers (x over the
4-core batch group, W over same-headgroup pairs) reassemble full
operands on chip, where links run ~1000x faster than the tunnel.

Per-core dataflow (all matmuls on fp16 operands, fp32 PSUM accum):
  P0: DRAM copies of the I/O shards -> internal tiles, AllGather x and W.
  P1: Q^T, K^T (dh-on-partitions, SBUF-resident) and V (tokens-on-
      partitions) projections from x^T.
  P2: per head: S^T = K_h^T^T@Q_h^T chunks -> exp (ScalarE, scaled
      1/sqrt(dh)) -> PV accumulation (attn^T in PSUM); DVE accumulates
      exp sums, a ones-matmul reduces+broadcasts the denominator across
      partitions, DVE reciprocal+multiply normalizes.
  P3: AllGather attn^T over the 4-core batch group, then each core
      computes its 512 output columns: out = attn_full @ Wo^T[:, cols].

Output per core: (2048 tokens, 512 out-cols) fp16; host concatenates
and casts to fp32.
"""

import math

import numpy as np

import concourse.bass as bass
import concourse.mybir as mybir
import concourse.tile as tile
from concourse import bacc
from concourse.bass_utils import run_bass_kernel_spmd

D = 2048
T = 2048
HG = 4  # heads per core
DH = 128
NI = 16  # contraction chunks of 128 over D
NQ = 4  # query-token chunks of 512
NT = 16  # token chunks of 128
SCALE = 1.0 / math.sqrt(DH)
F32 = mybir.dt.float32
F16 = mybir.dt.float16
GROUPS_BATCH = [[0, 1, 2, 3], [4, 5, 6, 7]]
GROUPS_PAIR = [[0, 4], [1, 5], [2, 6], [3, 7]]

_CACHED = {}


def build():
    nc = bacc.Bacc("TRN2", target_bir_lowering=False, debug=False, num_devices=8)
    # Disjoint fp16 shards: 512 rows of x[b]^T; 256-wide column slices of
    # each of Wq^T|Wk^T|Wv^T|Wo^T packed side by side.
    xTs = nc.declare_dram_parameter("xTs", [512, T], F16, isOutput=False)
    wAll = nc.declare_dram_parameter("wAll", [D, 4 * 256], F16, isOutput=False)
    out = nc.declare_dram_parameter("out", [T, HG * DH], F16, isOutput=True)

    with tile.TileContext(nc) as tc:
        with (
            nc.allow_low_precision(reason="fp16 storage; tolerance is 2e-2"),
            tc.tile_pool(name="dram", bufs=1, space="DRAM") as dram,
            tc.tile_pool(name="keep", bufs=1) as keep,
        ):
            x_int = dram.tile([512, T], F16)
            xT_full = dram.tile([D, T], F16)
            w_int = dram.tile([D, 4 * 256], F16)
            w_full = dram.tile([2 * D, 4 * 256], F16)
            attn_mine = dram.tile([HG * DH, T], F16)
            attn_all = dram.tile([4 * HG * DH, T], F16)

            v_sb = keep.tile([128, NT, HG * DH], F16)  # V: [tok128, tchunk, hdims]
            qT_sb = keep.tile([128, HG, T], F16)  # Q^T per head: [dh, head, tok]
            kT_sb = keep.tile([128, HG, T], F16)
            ones128 = keep.tile([128, 128], F16)
            nc.vector.memset(ones128[:], 1.0)

            # ------------- Phase 0: stage + AllGather inputs -------------
            nc.sync.dma_start(out=x_int[:], in_=xTs[:])
            nc.sync.dma_start(out=w_int[:], in_=wAll[:])
            nc.gpsimd.collective_compute(
                "AllGather",
                mybir.AluOpType.bypass,
                replica_groups=GROUPS_PAIR,
                ins=[w_int.opt()],
                outs=[w_full.opt()],
            )
            nc.gpsimd.collective_compute(
                "AllGather",
                mybir.AluOpType.bypass,
                replica_groups=GROUPS_BATCH,
                ins=[x_int.opt()],
                outs=[xT_full.opt()],
            )

            # ---------------- Phase 1: QKV projections ----------------
            with (
                tc.tile_pool(name="p1x", bufs=1) as p1x,
                tc.tile_pool(name="p1w", bufs=2) as p1w,
                tc.tile_pool(name="p1p", bufs=4, space="PSUM") as p1p,
            ):
                x_sb = p1x.tile([128, NI, T], F16)  # x^T resident: 64KB/part
                for i in range(NI):
                    nc.sync.dma_start(
                        out=x_sb[:, i, :], in_=xT_full[i * 128 : (i + 1) * 128, :]
                    )

                def load_w(widx):
                    # Reassemble [128, NI, 512] from the two gathered halves.
                    w_sb = p1w.tile([128, NI, HG * DH], F16, name="w_sb", tag="w_sb")
                    cs = slice(widx * 256, (widx + 1) * 256)
                    for i in range(NI):
                        nc.sync.dma_start(
                            out=w_sb[:, i, 0:256],
                            in_=w_full[i * 128 : (i + 1) * 128, cs],
                        )
                        nc.sync.dma_start(
                            out=w_sb[:, i, 256:512],
                            in_=w_full[D + i * 128 : D + (i + 1) * 128, cs],
                        )
                    return w_sb

                # Q^T and K^T: out rows = head dims (M), moving = tokens
                for widx, dst in ((0, qT_sb), (1, kT_sb)):
                    w_sb = load_w(widx)
                    for m in range(HG):
                        psums = []
                        for t in range(NQ):
                            psums.append(
                                p1p.tile([128, 512], F32, name="qk_ps", tag="qk_ps")
                            )
                        for i in range(NI):
                            lhsT = w_sb[:, i, m * 128 : (m + 1) * 128]
                            for t in range(NQ):
                                nc.tensor.matmul(
                                    psums[t][:],
                                    lhsT,
                                    x_sb[:, i, t * 512 : (t + 1) * 512],
                                    start=(i == 0),
                                    stop=(i == NI - 1),
                                )
                        for t in range(NQ):
                            nc.vector.tensor_copy(
                                dst[:, m, t * 512 : (t + 1) * 512], psums[t][:]
                            )

                # V: natural layout, tokens = M (stationary = x^T chunk)
                w_sb = load_w(2)
                for tc_i in range(NT):
                    ps = p1p.tile([128, 512], F32, name="v_ps", tag="v_ps")
                    for i in range(NI):
                        nc.tensor.matmul(
                            ps[:],
                            x_sb[:, i, tc_i * 128 : (tc_i + 1) * 128],
                            w_sb[:, i, :],
                            start=(i == 0),
                            stop=(i == NI - 1),
                        )
                    nc.vector.tensor_copy(v_sb[:, tc_i, :], ps[:])

            # ---------------- Phase 2: attention per head ----------------
            with (
                tc.tile_pool(name="p2e", bufs=4) as p2e,
                tc.tile_pool(name="p2a", bufs=2) as p2a,
                tc.tile_pool(name="p2n", bufs=2) as p2n,
                tc.tile_pool(name="p2ps", bufs=3, space="PSUM") as p2ps,
                tc.tile_pool(name="p2pa", bufs=2, space="PSUM") as p2pa,
                tc.tile_pool(name="p2pc", bufs=2, space="PSUM") as p2pc,
            ):
                for h in range(HG):
                    qh = qT_sb[:, h, :]
                    kh = kT_sb[:, h, :]
                    for q in range(NQ):
                        acc = p2a.tile([128, 512], F16, tag="acc")
                        attn_ps = p2pa.tile([128, 512], F32, tag="attn_ps")
                        for k in range(NT):
                            s_ps = p2ps.tile([128, 512], F32, tag="s_ps")
                            nc.tensor.matmul(
                                s_ps[:],
                                kh[:, k * 128 : (k + 1) * 128],
                                qh[:, q * 512 : (q + 1) * 512],
                            )
                            expS = p2e.tile([128, 512], F16, tag="expS")
                            nc.scalar.activation(
                                expS[:],
                                s_ps[:],
                                mybir.ActivationFunctionType.Exp,
                                scale=SCALE,
                            )
                            if k == 0:
                                nc.vector.tensor_copy(acc[:], expS[:])
                            else:
                                nc.vector.tensor_add(acc[:], acc[:], expS[:])
                            nc.tensor.matmul(
                                attn_ps[:],
                                v_sb[:, k, h * 128 : (h + 1) * 128],
                                expS[:],
                                start=(k == 0),
                                stop=(k == NT - 1),
                            )
                        # Reduce exp sums across partitions AND broadcast the
                        # denominator to all 128 partitions in one matmul.
                        bc_ps = p2pc.tile([128, 512], F32, tag="bc_ps")
                        nc.tensor.matmul(bc_ps[:], ones128[:], acc[:])
                        recip = p2n.tile([128, 512], F16, tag="recip")
                        nc.vector.reciprocal(recip[:], bc_ps[:])
                        attn_sb = p2a.tile([128, 512], F16, tag="attn_sb")
                        nc.vector.tensor_mul(attn_sb[:], attn_ps[:], recip[:])
                        nc.sync.dma_start(
                            out=attn_mine[
                                h * 128 : (h + 1) * 128, q * 512 : (q + 1) * 512
                            ],
                            in_=attn_sb[:],
                        )

            # ---------------- AllGather over batch group ----------------
            nc.gpsimd.collective_compute(
                "AllGather",
                mybir.AluOpType.bypass,
                replica_groups=GROUPS_BATCH,
                ins=[attn_mine.opt()],
                outs=[attn_all.opt()],
            )

            # ---------------- Phase 3: output projection ----------------
            with (
                tc.tile_pool(name="p3w", bufs=1) as p3w,
                tc.tile_pool(name="p3a", bufs=8) as p3a,
                tc.tile_pool(name="p3o", bufs=4) as p3o,
                tc.tile_pool(name="p3p", bufs=4, space="PSUM") as p3p,
            ):
                wo_sb = p3w.tile([128, NI, HG * DH], F16)
                cs = slice(3 * 256, 4 * 256)
                for i in range(NI):
                    nc.sync.dma_start(
                        out=wo_sb[:, i, 0:256],
                        in_=w_full[i * 128 : (i + 1) * 128, cs],
                    )
                    nc.sync.dma_start(
                        out=wo_sb[:, i, 256:512],
                        in_=w_full[D + i * 128 : D + (i + 1) * 128, cs],
                    )
                for t in range(NT):
                    ps = p3p.tile([128, 512], F32)
                    for i in range(NI):
                        a_tile = p3a.tile([128, 128], F16, tag="a_tile")
                        nc.sync.dma_start(
                            out=a_tile[:],
                            in_=attn_all[
                                i * 128 : (i + 1) * 128, t * 128 : (t + 1) * 128
                            ],
                        )
                        nc.tensor.matmul(
                            ps[:],
                            a_tile[:],
                            wo_sb[:, i, :],
                            start=(i == 0),
                            stop=(i == NI - 1),
                        )
                    o_sb = p3o.tile([128, 512], F16)
                    nc.vector.tensor_copy(o_sb[:], ps[:])
                    nc.sync.dma_start(
                        out=out[t * 128 : (t + 1) * 128, :], in_=o_sb[:]
                    )

    nc.compile()
    return nc


def _get_nc():
    if "nc" not in _CACHED:
        _CACHED["nc"] = build()
    return _CACHED["nc"]


def kernel(x, Wq, Wk, Wv, Wo, _trace=False):
    x = np.asarray(x)
    B = x.shape[0]

    x16 = [np.asarray(x[b], dtype=np.float16) for b in range(B)]
    w16T = [np.asarray(W, dtype=np.float16).T for W in (Wq, Wk, Wv, Wo)]

    in_maps = []
    for c in range(8):
        b, r = divmod(c, 4)
        half = c // 4
        xsl = slice(r * 512, (r + 1) * 512)
        wsl = slice(r * 512 + half * 256, r * 512 + half * 256 + 256)
        in_maps.append(
            {
                "xTs": np.ascontiguousarray(x16[b][:, xsl].T),
                "wAll": np.concatenate([w[:, wsl] for w in w16T], axis=1),
            }
        )

    nc = _get_nc()
    res = run_bass_kernel_spmd(nc, in_maps, list(range(8)), trace=_trace)
    _CACHED["last_result"] = res

    out = np.empty((B, T, D), dtype=np.float32)
    for c in range(8):
        b, r = divmod(c, 4)
        out[b, :, r * 512 : (r + 1) * 512] = res.results[c]["out"]
    return out


# revision 5
# speedup vs baseline: 6.0502x; 1.0470x over previous
"""Multi-head attention (B=2, T=2048, D=2048, 16 heads) on 8 NeuronCores.

Sharding: DP=2 over batch x TP=4 over heads (4 heads/core).
Core c handles batch b=c//4, head group r=c%4 (heads 4r..4r+3).

Host->device traffic is the bottleneck (axon tunnel ~60MB/s up, ~30MB/s
down), so inputs are shipped fp16 and fully deduplicated: each core
uploads a disjoint 1/8th of x (512 rows of x[b]^T) and disjoint quarter
column-slices of each weight.  Two on-device AllGathers (x over the
4-core batch group, W over same-headgroup pairs) reassemble full
operands on chip, where links run ~1000x faster than the tunnel.

Per-core dataflow (all matmuls on fp16 operands, fp32 PSUM accum):
  P0: DRAM copies of the I/O shards -> internal tiles, AllGather x and W.
  P1: Q^T, K^T (dh-on-partitions, SBUF-resident) and V (tokens-on-
      partitions) projections from x^T.
  P2: per head: S^T = K_h^T^T@Q_h^T chunks -> exp (ScalarE, scaled
      1/sqrt(dh)) -> PV accumulation (attn^T in PSUM); DVE accumulates
      exp sums, a ones-matmul reduces+broadcasts the denominator across
      partitions, DVE reciprocal+multiply normalizes.
  P3: AllGather attn^T over the 4-core batch group, then each core
      computes its 512 output columns: out = attn_full @ Wo^T[:, cols].

Output per core: (2048 tokens, 512 out-cols) fp16; host concatenates
and casts to fp32.
"""

import math

import numpy as np

import concourse.bass as bass
import concourse.mybir as mybir
import concourse.tile as tile
from concourse import bacc
from concourse.bass_utils import run_bass_kernel_spmd

D = 2048
T = 2048
HG = 4  # heads per core
DH = 128
NI = 16  # contraction chunks of 128 over D
NQ = 4  # query-token chunks of 512
NT = 16  # token chunks of 128
SCALE = 1.0 / math.sqrt(DH)
F32 = mybir.dt.float32
F16 = mybir.dt.float16
GROUPS_BATCH = [[0, 1, 2, 3], [4, 5, 6, 7]]
GROUPS_PAIR = [[0, 4], [1, 5], [2, 6], [3, 7]]

_CACHED = {}


def build():
    nc = bacc.Bacc("TRN2", target_bir_lowering=False, debug=False, num_devices=8)
    # Disjoint fp16 shards: 512 rows of x[b]^T; 256-wide column slices of
    # each of Wq^T|Wk^T|Wv^T|Wo^T packed side by side.
    xTs = nc.declare_dram_parameter("xTs", [512, T], F16, isOutput=False)
    wAll = nc.declare_dram_parameter("wAll", [D, 4 * 256], F16, isOutput=False)
    out = nc.declare_dram_parameter("out", [T, HG * DH], F16, isOutput=True)

    with tile.TileContext(nc) as tc:
        with (
            nc.allow_low_precision(reason="fp16 storage; tolerance is 2e-2"),
            tc.tile_pool(name="dram", bufs=1, space="DRAM") as dram,
            tc.tile_pool(name="keep", bufs=1) as keep,
        ):
            x_int = dram.tile([512, T], F16)
            xT_full = dram.tile([D, T], F16)
            w_int = dram.tile([D, 4 * 256], F16)
            w_full = dram.tile([2 * D, 4 * 256], F16)
            attn_mine = dram.tile([HG * DH, T], F16)
            attn_all = dram.tile([4 * HG * DH, T], F16)

            v_sb = keep.tile([128, NT, HG * DH], F16)  # V: [tok128, tchunk, hdims]
            qT_sb = keep.tile([128, HG, T], F16)  # Q^T per head: [dh, head, tok]
            kT_sb = keep.tile([128, HG, T], F16)
            ones128 = keep.tile([128, 128], F16)
            nc.vector.memset(ones128[:], 1.0)

            # ------------- Phase 0: stage + AllGather inputs -------------
            nc.sync.dma_start(out=x_int[:], in_=xTs[:])
            nc.sync.dma_start(out=w_int[:], in_=wAll[:])
            nc.gpsimd.collective_compute(
                "AllGather",
                mybir.AluOpType.bypass,
                replica_groups=GROUPS_PAIR,
                ins=[w_int.opt()],
                outs=[w_full.opt()],
            )
            nc.gpsimd.collective_compute(
                "AllGather",
                mybir.AluOpType.bypass,
                replica_groups=GROUPS_BATCH,
                ins=[x_int.opt()],
                outs=[xT_full.opt()],
            )

            # ---------------- Phase 1: QKV projections ----------------
            with (
                tc.tile_pool(name="p1x", bufs=1) as p1x,
                tc.tile_pool(name="p1w", bufs=2) as p1w,
                tc.tile_pool(name="p1p", bufs=4, space="PSUM") as p1p,
            ):
                x_sb = p1x.tile([128, NI, T], F16)  # x^T resident: 64KB/part
                for i in range(NI):
                    nc.sync.dma_start(
                        out=x_sb[:, i, :], in_=xT_full[i * 128 : (i + 1) * 128, :]
                    )

                def load_w(widx):
                    # Reassemble [128, NI, 512] from the two gathered halves.
                    w_sb = p1w.tile([128, NI, HG * DH], F16, name="w_sb", tag="w_sb")
                    cs = slice(widx * 256, (widx + 1) * 256)
                    for i in range(NI):
                        nc.sync.dma_start(
                            out=w_sb[:, i, 0:256],
                            in_=w_full[i * 128 : (i + 1) * 128, cs],
                        )
                        nc.sync.dma_start(
                            out=w_sb[:, i, 256:512],
                            in_=w_full[D + i * 128 : D + (i + 1) * 128, cs],
                        )
                    return w_sb

                # Q^T and K^T: out rows = head dims (M), moving = tokens
                for widx, dst in ((0, qT_sb), (1, kT_sb)):
                    w_sb = load_w(widx)
                    for m in range(HG):
                        psums = []
                        for t in range(NQ):
                            psums.append(
                                p1p.tile([128, 512], F32, name="qk_ps", tag="qk_ps")
                            )
                        for i in range(NI):
                            lhsT = w_sb[:, i, m * 128 : (m + 1) * 128]
                            for t in range(NQ):
                                nc.tensor.matmul(
                                    psums[t][:],
                                    lhsT,
                                    x_sb[:, i, t * 512 : (t + 1) * 512],
                                    start=(i == 0),
                                    stop=(i == NI - 1),
                                )
                        for t in range(NQ):
                            nc.vector.tensor_copy(
                                dst[:, m, t * 512 : (t + 1) * 512], psums[t][:]
                            )

                # V: natural layout, tokens = M (stationary = x^T chunk)
                w_sb = load_w(2)
                for tc_i in range(NT):
                    ps = p1p.tile([128, 512], F32, name="v_ps", tag="v_ps")
                    for i in range(NI):
                        nc.tensor.matmul(
                            ps[:],
                            x_sb[:, i, tc_i * 128 : (tc_i + 1) * 128],
                            w_sb[:, i, :],
                            start=(i == 0),
                            stop=(i == NI - 1),
                        )
                    nc.vector.tensor_copy(v_sb[:, tc_i, :], ps[:])

            # ---------------- Phase 2: attention per head ----------------
            with (
                tc.tile_pool(name="p2e", bufs=4) as p2e,
                tc.tile_pool(name="p2a", bufs=2) as p2a,
                tc.tile_pool(name="p2n", bufs=2) as p2n,
                tc.tile_pool(name="p2ps", bufs=3, space="PSUM") as p2ps,
                tc.tile_pool(name="p2pa", bufs=2, space="PSUM") as p2pa,
                tc.tile_pool(name="p2pc", bufs=2, space="PSUM") as p2pc,
            ):
                for h in range(HG):
                    qh = qT_sb[:, h, :]
                    kh = kT_sb[:, h, :]
                    for q in range(NQ):
                        acc = p2a.tile([128, 512], F16, tag="acc")
                        attn_ps = p2pa.tile([128, 512], F32, tag="attn_ps")
                        for k in range(NT):
                            s_ps = p2ps.tile([128, 512], F32, tag="s_ps")
                            nc.tensor.matmul(
                                s_ps[:],
                                kh[:, k * 128 : (k + 1) * 128],
                                qh[:, q * 512 : (q + 1) * 512],
                            )
                            expS = p2e.tile([128, 512], F16, tag="expS")
                            nc.scalar.activation(
                                expS[:],
                                s_ps[:],
                                mybir.ActivationFunctionType.Exp,
                                scale=SCALE,
                            )
                            if k == 0:
                                nc.vector.tensor_copy(acc[:], expS[:])
                            else:
                                nc.vector.tensor_add(acc[:], acc[:], expS[:])
                            nc.tensor.matmul(
                                attn_ps[:],
                                v_sb[:, k, h * 128 : (h + 1) * 128],
                                expS[:],
                                start=(k == 0),
                                stop=(k == NT - 1),
                            )
                        # Reduce exp sums across partitions AND broadcast the
                        # denominator to all 128 partitions in one matmul.
                        bc_ps = p2pc.tile([128, 512], F32, tag="bc_ps")
                        nc.tensor.matmul(bc_ps[:], ones128[:], acc[:])
                        recip = p2n.tile([128, 512], F16, tag="recip")
                        nc.vector.reciprocal(recip[:], bc_ps[:])
                        attn_sb = p2a.tile([128, 512], F16, tag="attn_sb")
                        nc.vector.tensor_mul(attn_sb[:], attn_ps[:], recip[:])
                        nc.sync.dma_start(
                            out=attn_mine[
                                h * 128 : (h + 1) * 128, q * 512 : (q + 1) * 512
                            ],
                            in_=attn_sb[:],
                        )

            # ---------------- AllGather over batch group ----------------
            nc.gpsimd.collective_compute(
                "AllGather",
                mybir.AluOpType.bypass,
                replica_groups=GROUPS_BATCH,
                ins=[attn_mine.opt()],
                outs=[attn_all.opt()],
            )

            # ---------------- Phase 3: output projection ----------------
            with (
                tc.tile_pool(name="p3w", bufs=1) as p3w,
                tc.tile_pool(name="p3a", bufs=8) as p3a,
                tc.tile_pool(name="p3o", bufs=4) as p3o,
                tc.tile_pool(name="p3p", bufs=4, space="PSUM") as p3p,
            ):
                wo_sb = p3w.tile([128, NI, HG * DH], F16)
                cs = slice(3 * 256, 4 * 256)
                for i in range(NI):
                    nc.sync.dma_start(
                        out=wo_sb[:, i, 0:256],
                        in_=w_full[i * 128 : (i + 1) * 128, cs],
                    )
                    nc.sync.dma_start(
                        out=wo_sb[:, i, 256:512],
                        in_=w_full[D + i * 128 : D + (i + 1) * 128, cs],
                    )
                for t in range(NT):
                    ps = p3p.tile([128, 512], F32)
                    for i in range(NI):
                        a_tile = p3a.tile([128, 128], F16, tag="a_tile")
                        nc.sync.dma_start(
                            out=a_tile[:],
                            in_=attn_all[
                                i * 128 : (i + 1) * 128, t * 128 : (t + 1) * 128
                            ],
                        )
                        nc.tensor.matmul(
                            ps[:],
                            a_tile[:],
                            wo_sb[:, i, :],
                            start=(i == 0),
                            stop=(i == NI - 1),
                        )
                    o_sb = p3o.tile([128, 512], F16)
                    nc.vector.tensor_copy(o_sb[:], ps[:])
                    nc.sync.dma_start(
                        out=out[t * 128 : (t + 1) * 128, :], in_=o_sb[:]
                    )

    nc.compile()
    return nc


def _get_nc():
    if "nc" not in _CACHED:
        _CACHED["nc"] = build()
    return _CACHED["nc"]


def _build_shards(x, Wq, Wk, Wv, Wo):
    """Global (concatenated-over-cores) fp16 input arrays."""
    x = np.asarray(x)
    # X rows c*512..: core c=(b,r) gets rows r*512:(r+1)*512 of x[b]^T,
    # i.e. X.reshape(2,2048,2048)[b] == x[b].T.
    X = np.empty((8 * 512, T), dtype=np.float16)
    Xv = X.reshape(2, D, T)
    for b in range(2):
        Xv[b] = np.asarray(x[b]).T
    W = np.empty((8 * D, 4 * 256), dtype=np.float16)
    w16T = [np.asarray(Wm, dtype=np.float16).T for Wm in (Wq, Wk, Wv, Wo)]
    for c in range(8):
        r, half = c % 4, c // 4
        wsl = slice(r * 512 + half * 256, r * 512 + half * 256 + 256)
        for widx, w in enumerate(w16T):
            W[c * D : (c + 1) * D, widx * 256 : (widx + 1) * 256] = w[:, wsl]
    return X, W


def _get_runner():
    if "runner" in _CACHED:
        return _CACHED["runner"]

    import jax
    import jax.numpy as jnp
    from jax.sharding import Mesh, NamedSharding, PartitionSpec

    try:
        from jax import shard_map
    except ImportError:
        from jax.experimental.shard_map import shard_map
    from concourse.bass2jax import (
        _bass_exec_p,
        install_neuronx_cc_hook,
        partition_id_tensor,
    )

    install_neuronx_cc_hook()
    nc = _get_nc()

    partition_name = nc.partition_id_tensor.name if nc.partition_id_tensor else None
    in_names, out_names, out_avals = [], [], []
    for alloc in nc.m.functions[0].allocations:
        if not isinstance(alloc, mybir.MemoryLocationSet):
            continue
        name = alloc.memorylocations[0].name
        if alloc.kind == "ExternalInput":
            if name != partition_name:
                in_names.append(name)
        elif alloc.kind == "ExternalOutput":
            out_names.append(name)
            out_avals.append(
                jax.core.ShapedArray(tuple(alloc.tensor_shape), mybir.dt.np(alloc.dtype))
            )
    n_params = len(in_names)
    all_names = in_names + out_names + ([partition_name] if partition_name else [])
    donate = tuple(range(n_params, n_params + len(out_names)))

    def _body(*args):
        operands = list(args)
        if partition_name is not None:
            operands.append(partition_id_tensor())
        return tuple(
            _bass_exec_p.bind(
                *operands,
                out_avals=tuple(out_avals),
                in_names=tuple(all_names),
                out_names=tuple(out_names),
                lowering_input_output_aliases=(),
                sim_require_finite=True,
                sim_require_nnan=True,
                nc=nc,
            )
        )

    devices = jax.devices()[:8]
    mesh = Mesh(np.asarray(devices), ("core",))
    spec = PartitionSpec("core")
    nshard = NamedSharding(mesh, spec)
    n_io = n_params + len(out_names)
    smap_kw = dict(mesh=mesh, in_specs=(spec,) * n_io, out_specs=(spec,) * len(out_names))
    try:
        smapped = shard_map(_body, check_vma=False, **smap_kw)
    except TypeError:
        smapped = shard_map(_body, check_rep=False, **smap_kw)
    sharded = jax.jit(smapped, donate_argnums=donate, keep_unused=True)
    zero_shapes = [(8 * a.shape[0], *a.shape[1:]) for a in out_avals]
    zero_dtypes = [a.dtype for a in out_avals]
    zeros_fn = jax.jit(
        lambda: tuple(
            jnp.zeros(s, d) for s, d in zip(zero_shapes, zero_dtypes)
        ),
        out_shardings=(nshard,) * len(out_names),
    )

    def run(X, W):
        zeros = zeros_fn()  # created on-device: no tunnel bytes
        X_dev = jax.device_put(X, nshard)  # start upload before W is passed
        out_arrs = sharded(X_dev, W, *zeros)
        return [np.asarray(o) for o in out_arrs]

    _CACHED["runner"] = run
    return run


def kernel(x, Wq, Wk, Wv, Wo, _trace=False):
    x = np.asarray(x)
    B = x.shape[0]
    X, W = _build_shards(x, Wq, Wk, Wv, Wo)

    if _trace:
        in_maps = [
            {"xTs": X[c * 512 : (c + 1) * 512], "wAll": W[c * D : (c + 1) * D]}
            for c in range(8)
        ]
        res = run_bass_kernel_spmd(_get_nc(), in_maps, list(range(8)), trace=True)
        _CACHED["last_result"] = res
        outs = [res.results[c]["out"] for c in range(8)]
    else:
        out_global = _get_runner()(X, W)[0]
        outs = out_global.reshape(8, T, HG * DH)

    out = np.empty((B, T, D), dtype=np.float32)
    for c in range(8):
        b, r = divmod(c, 4)
        out[b, :, r * 512 : (r + 1) * 512] = outs[c]
    return out


# revision 13
# speedup vs baseline: 10.8706x; 1.7967x over previous
"""Multi-head attention (B=2, T=2048, D=2048, 16 heads) on 8 NeuronCores.

Sharding: DP=2 over batch x TP=4 over heads (4 heads/core).
Core c handles batch b=c//4, head group r=c%4 (heads 4r..4r+3).

Host->device traffic is the bottleneck (axon tunnel ~60MB/s up, ~30MB/s
down), so inputs are shipped fp16 and fully deduplicated: each core
uploads a disjoint 1/8th of x (512 rows of x[b]^T) and disjoint quarter
column-slices of each weight.  Two on-device AllGathers (x over the
4-core batch group, W over same-headgroup pairs) reassemble full
operands on chip, where links run ~1000x faster than the tunnel.

Per-core dataflow (all matmuls on fp16 operands, fp32 PSUM accum):
  P0: DRAM copies of the I/O shards -> internal tiles, AllGather x and W.
  P1: Q^T, K^T (dh-on-partitions, SBUF-resident) and V (tokens-on-
      partitions) projections from x^T.
  P2: per head: S^T = K_h^T^T@Q_h^T chunks -> exp (ScalarE, scaled
      1/sqrt(dh)) -> PV accumulation (attn^T in PSUM); DVE accumulates
      exp sums, a ones-matmul reduces+broadcasts the denominator across
      partitions, DVE reciprocal+multiply normalizes.
  P3: AllGather attn^T over the 4-core batch group, then each core
      computes its 512 output columns: out = attn_full @ Wo^T[:, cols].

Output per core: (2048 tokens, 512 out-cols) fp16; host concatenates
and casts to fp32.
"""

import math

import numpy as np

import concourse.bass as bass
import concourse.mybir as mybir
import concourse.tile as tile
from concourse import bacc
from concourse.bass_utils import run_bass_kernel_spmd

D = 2048
T = 2048
HG = 4  # heads per core
DH = 128
NI = 16  # contraction chunks of 128 over D
NQ = 4  # query-token chunks of 512
NT = 16  # token chunks of 128
SCALE = 1.0 / math.sqrt(DH)
F32 = mybir.dt.float32
F16 = mybir.dt.float16
GROUPS_BATCH = [[0, 1, 2, 3], [4, 5, 6, 7]]
GROUPS_PAIR = [[0, 4], [1, 5], [2, 6], [3, 7]]

_CACHED = {}


def build():
    nc = bacc.Bacc("TRN2", target_bir_lowering=False, debug=False, num_devices=8)
    # Disjoint fp16 shards in NATURAL row layout (host does casts only, no
    # transposes; the device transposes via DMA-XBAR on SBUF load):
    # xNat = x[b][r*512:(r+1)*512, :]; wNat = 256-row slices of each of
    # Wq|Wk|Wv|Wo stacked.
    xNat = nc.declare_dram_parameter("xNat", [512, D], F16, isOutput=False)
    wNat = nc.declare_dram_parameter("wNat", [4 * 256, D], F16, isOutput=False)
    out = nc.declare_dram_parameter("out", [T, HG * DH], F16, isOutput=True)

    with tile.TileContext(nc) as tc:
        with (
            nc.allow_low_precision(reason="fp16 storage; tolerance is 2e-2"),
            tc.tile_pool(name="dram", bufs=1, space="DRAM") as dram,
            tc.tile_pool(name="keep", bufs=1) as keep,
        ):
            x_int = dram.tile([512, D], F16)
            x_full = dram.tile([T, D], F16)
            w_int = dram.tile([4 * 256, D], F16)
            w_full = dram.tile([2 * 4 * 256, D], F16)
            attn_mine = dram.tile([HG * DH, T], F16)
            attn_all = dram.tile([4 * HG * DH, T], F16)

            v_sb = keep.tile([128, NT, HG * DH], F16)  # V: [tok128, tchunk, hdims]
            qT_sb = keep.tile([128, HG, T], F16)  # Q^T per head: [dh, head, tok]
            kT_sb = keep.tile([128, HG, T], F16)
            ones128 = keep.tile([128, 128], F16)
            nc.vector.memset(ones128[:], 1.0)

            # ------------- Phase 0: stage + AllGather inputs -------------
            nc.sync.dma_start(out=x_int[:], in_=xNat[:])
            nc.sync.dma_start(out=w_int[:], in_=wNat[:])
            nc.gpsimd.collective_compute(
                "AllGather",
                mybir.AluOpType.bypass,
                replica_groups=GROUPS_PAIR,
                ins=[w_int.opt()],
                outs=[w_full.opt()],
            )
            nc.gpsimd.collective_compute(
                "AllGather",
                mybir.AluOpType.bypass,
                replica_groups=GROUPS_BATCH,
                ins=[x_int.opt()],
                outs=[x_full.opt()],
            )

            # ---------------- Phase 1: QKV projections ----------------
            with (
                tc.tile_pool(name="p1x", bufs=1) as p1x,
                tc.tile_pool(name="p1w", bufs=2) as p1w,
                tc.tile_pool(name="p1p", bufs=4, space="PSUM") as p1p,
            ):
                x_sb = p1x.tile([128, NI, T], F16)  # x^T resident: 64KB/part
                for i in range(NI):
                    for t in range(NQ):
                        nc.sync.dma_start_transpose(
                            out=x_sb[:, i, t * 512 : (t + 1) * 512],
                            in_=x_full[
                                t * 512 : (t + 1) * 512, i * 128 : (i + 1) * 128
                            ],
                        )

                def load_w(widx):
                    # Reassemble W^T [128, NI, 512] from the two gathered
                    # natural-layout halves via transposing DMA.
                    w_sb = p1w.tile([128, NI, HG * DH], F16, name="w_sb", tag="w_sb")
                    rs0 = widx * 256
                    rs1 = 4 * 256 + widx * 256
                    for i in range(NI):
                        nc.sync.dma_start_transpose(
                            out=w_sb[:, i, 0:256],
                            in_=w_full[rs0 : rs0 + 256, i * 128 : (i + 1) * 128],
                        )
                        nc.sync.dma_start_transpose(
                            out=w_sb[:, i, 256:512],
                            in_=w_full[rs1 : rs1 + 256, i * 128 : (i + 1) * 128],
                        )
                    return w_sb

                # Q^T and K^T: out rows = head dims (M), moving = tokens
                for widx, dst in ((0, qT_sb), (1, kT_sb)):
                    w_sb = load_w(widx)
                    for m in range(HG):
                        psums = []
                        for t in range(NQ):
                            psums.append(
                                p1p.tile([128, 512], F32, name="qk_ps", tag="qk_ps")
                            )
                        for i in range(NI):
                            lhsT = w_sb[:, i, m * 128 : (m + 1) * 128]
                            for t in range(NQ):
                                nc.tensor.matmul(
                                    psums[t][:],
                                    lhsT,
                                    x_sb[:, i, t * 512 : (t + 1) * 512],
                                    start=(i == 0),
                                    stop=(i == NI - 1),
                                )
                        for t in range(NQ):
                            nc.vector.tensor_copy(
                                dst[:, m, t * 512 : (t + 1) * 512], psums[t][:]
                            )

                # V: natural layout, tokens = M (stationary = x^T chunk)
                w_sb = load_w(2)
                for tc_i in range(NT):
                    ps = p1p.tile([128, 512], F32, name="v_ps", tag="v_ps")
                    for i in range(NI):
                        nc.tensor.matmul(
                            ps[:],
                            x_sb[:, i, tc_i * 128 : (tc_i + 1) * 128],
                            w_sb[:, i, :],
                            start=(i == 0),
                            stop=(i == NI - 1),
                        )
                    nc.vector.tensor_copy(v_sb[:, tc_i, :], ps[:])

            # ---------------- Phase 2: attention per head ----------------
            with (
                tc.tile_pool(name="p2e", bufs=4) as p2e,
                tc.tile_pool(name="p2a", bufs=2) as p2a,
                tc.tile_pool(name="p2n", bufs=2) as p2n,
                tc.tile_pool(name="p2ps", bufs=3, space="PSUM") as p2ps,
                tc.tile_pool(name="p2pa", bufs=2, space="PSUM") as p2pa,
                tc.tile_pool(name="p2pc", bufs=2, space="PSUM") as p2pc,
            ):
                for h in range(HG):
                    qh = qT_sb[:, h, :]
                    kh = kT_sb[:, h, :]
                    for q in range(NQ):
                        acc = p2a.tile([128, 512], F16, tag="acc")
                        attn_ps = p2pa.tile([128, 512], F32, tag="attn_ps")
                        for k in range(NT):
                            s_ps = p2ps.tile([128, 512], F32, tag="s_ps")
                            nc.tensor.matmul(
                                s_ps[:],
                                kh[:, k * 128 : (k + 1) * 128],
                                qh[:, q * 512 : (q + 1) * 512],
                            )
                            expS = p2e.tile([128, 512], F16, tag="expS")
                            nc.scalar.activation(
                                expS[:],
                                s_ps[:],
                                mybir.ActivationFunctionType.Exp,
                                scale=SCALE,
                            )
                            if k == 0:
                                nc.vector.tensor_copy(acc[:], expS[:])
                            else:
                                nc.vector.tensor_add(acc[:], acc[:], expS[:])
                            nc.tensor.matmul(
                                attn_ps[:],
                                v_sb[:, k, h * 128 : (h + 1) * 128],
                                expS[:],
                                start=(k == 0),
                                stop=(k == NT - 1),
                            )
                        # Reduce exp sums across partitions AND broadcast the
                        # denominator to all 128 partitions in one matmul.
                        bc_ps = p2pc.tile([128, 512], F32, tag="bc_ps")
                        nc.tensor.matmul(bc_ps[:], ones128[:], acc[:])
                        recip = p2n.tile([128, 512], F16, tag="recip")
                        nc.vector.reciprocal(recip[:], bc_ps[:])
                        attn_sb = p2a.tile([128, 512], F16, tag="attn_sb")
                        nc.vector.tensor_mul(attn_sb[:], attn_ps[:], recip[:])
                        nc.sync.dma_start(
                            out=attn_mine[
                                h * 128 : (h + 1) * 128, q * 512 : (q + 1) * 512
                            ],
                            in_=attn_sb[:],
                        )

            # ---------------- AllGather over batch group ----------------
            nc.gpsimd.collective_compute(
                "AllGather",
                mybir.AluOpType.bypass,
                replica_groups=GROUPS_BATCH,
                ins=[attn_mine.opt()],
                outs=[attn_all.opt()],
            )

            # ---------------- Phase 3: output projection ----------------
            with (
                tc.tile_pool(name="p3w", bufs=1) as p3w,
                tc.tile_pool(name="p3a", bufs=8) as p3a,
                tc.tile_pool(name="p3o", bufs=4) as p3o,
                tc.tile_pool(name="p3p", bufs=4, space="PSUM") as p3p,
            ):
                wo_sb = p3w.tile([128, NI, HG * DH], F16)
                rs0 = 3 * 256
                rs1 = 4 * 256 + 3 * 256
                for i in range(NI):
                    nc.sync.dma_start_transpose(
                        out=wo_sb[:, i, 0:256],
                        in_=w_full[rs0 : rs0 + 256, i * 128 : (i + 1) * 128],
                    )
                    nc.sync.dma_start_transpose(
                        out=wo_sb[:, i, 256:512],
                        in_=w_full[rs1 : rs1 + 256, i * 128 : (i + 1) * 128],
                    )
                for t in range(NT):
                    ps = p3p.tile([128, 512], F32)
                    for i in range(NI):
                        a_tile = p3a.tile([128, 128], F16, tag="a_tile")
                        nc.sync.dma_start(
                            out=a_tile[:],
                            in_=attn_all[
                                i * 128 : (i + 1) * 128, t * 128 : (t + 1) * 128
                            ],
                        )
                        nc.tensor.matmul(
                            ps[:],
                            a_tile[:],
                            wo_sb[:, i, :],
                            start=(i == 0),
                            stop=(i == NI - 1),
                        )
                    o_sb = p3o.tile([128, 512], F16)
                    nc.vector.tensor_copy(o_sb[:], ps[:])
                    nc.sync.dma_start(
                        out=out[t * 128 : (t + 1) * 128, :], in_=o_sb[:]
                    )

    nc.compile()
    return nc


def _get_nc():
    if "nc" not in _CACHED:
        _CACHED["nc"] = build()
    return _CACHED["nc"]


def _build_shards(x, Wq, Wk, Wv, Wo):
    """Global (concatenated-over-cores) fp16 input arrays.

    Natural row layout so this is pure dtype-cast streaming (no host
    transposes): core c=(b,r,half) gets x[b][r*512:(r+1)*512, :] and the
    rows [r*512+half*256 : +256) of each of Wq|Wk|Wv|Wo stacked.
    """
    x = np.asarray(x)
    X = np.empty((8 * 512, D), dtype=np.float16)
    X.reshape(2, T, D)[:] = x  # cast; core c rows = x[c//4][(c%4)*512:...]
    W = np.empty((8 * 4 * 256, D), dtype=np.float16)
    Wv4 = W.reshape(8, 4, 256, D)
    for c in range(8):
        r, half = c % 4, c // 4
        wsl = slice(r * 512 + half * 256, r * 512 + half * 256 + 256)
        for widx, Wm in enumerate((Wq, Wk, Wv, Wo)):
            Wv4[c, widx] = Wm[wsl, :]
    return X, W


def _get_runner():
    if "runner" in _CACHED:
        return _CACHED["runner"]

    import jax
    import jax.numpy as jnp
    from jax.sharding import Mesh, NamedSharding, PartitionSpec

    try:
        from jax import shard_map
    except ImportError:
        from jax.experimental.shard_map import shard_map
    from concourse.bass2jax import (
        _bass_exec_p,
        install_neuronx_cc_hook,
        partition_id_tensor,
    )

    install_neuronx_cc_hook()
    nc = _get_nc()

    partition_name = nc.partition_id_tensor.name if nc.partition_id_tensor else None
    in_names, out_names, out_avals = [], [], []
    for alloc in nc.m.functions[0].allocations:
        if not isinstance(alloc, mybir.MemoryLocationSet):
            continue
        name = alloc.memorylocations[0].name
        if alloc.kind == "ExternalInput":
            if name != partition_name:
                in_names.append(name)
        elif alloc.kind == "ExternalOutput":
            out_names.append(name)
            out_avals.append(
                jax.core.ShapedArray(tuple(alloc.tensor_shape), mybir.dt.np(alloc.dtype))
            )
    n_params = len(in_names)
    all_names = in_names + out_names + ([partition_name] if partition_name else [])
    donate = tuple(range(n_params, n_params + len(out_names)))

    def _body(*args):
        operands = list(args)
        if partition_name is not None:
            operands.append(partition_id_tensor())
        return tuple(
            _bass_exec_p.bind(
                *operands,
                out_avals=tuple(out_avals),
                in_names=tuple(all_names),
                out_names=tuple(out_names),
                lowering_input_output_aliases=(),
                sim_require_finite=True,
                sim_require_nnan=True,
                nc=nc,
            )
        )

    devices = jax.devices()[:8]
    mesh = Mesh(np.asarray(devices), ("core",))
    spec = PartitionSpec("core")
    nshard = NamedSharding(mesh, spec)
    n_io = n_params + len(out_names)
    smap_kw = dict(mesh=mesh, in_specs=(spec,) * n_io, out_specs=(spec,) * len(out_names))
    try:
        smapped = shard_map(_body, check_vma=False, **smap_kw)
    except TypeError:
        smapped = shard_map(_body, check_rep=False, **smap_kw)
    sharded = jax.jit(smapped, donate_argnums=donate, keep_unused=True)
    zero_shapes = [(8 * a.shape[0], *a.shape[1:]) for a in out_avals]
    zero_dtypes = [a.dtype for a in out_avals]
    zeros_fn = jax.jit(
        lambda: tuple(
            jnp.zeros(s, d) for s, d in zip(zero_shapes, zero_dtypes)
        ),
        out_shardings=(nshard,) * len(out_names),
    )

    def run(X, W):
        zeros = zeros_fn()  # created on-device: no tunnel bytes
        X_dev = jax.device_put(X, nshard)  # start upload before W is passed
        out_arrs = sharded(X_dev, W, *zeros)
        return [np.asarray(o) for o in out_arrs]

    _CACHED["runner"] = run
    return run


def kernel(x, Wq, Wk, Wv, Wo, _trace=False):
    x = np.asarray(x)
    B = x.shape[0]
    X, W = _build_shards(x, Wq, Wk, Wv, Wo)

    if _trace:
        in_maps = [
            {
                "xNat": X[c * 512 : (c + 1) * 512],
                "wNat": W[c * 1024 : (c + 1) * 1024],
            }
            for c in range(8)
        ]
        res = run_bass_kernel_spmd(_get_nc(), in_maps, list(range(8)), trace=True)
        _CACHED["last_result"] = res
        outs = [res.results[c]["out"] for c in range(8)]
    else:
        out_global = _get_runner()(X, W)[0]
        outs = out_global.reshape(8, T, HG * DH)

    out = np.empty((B, T, D), dtype=np.float32)
    for c in range(8):
        b, r = divmod(c, 4)
        out[b, :, r * 512 : (r + 1) * 512] = outs[c]
    return out
